# revision 5
# baseline (speedup 1.0000x reference)
"""Causal multi-head attention (B=1, S=4096, H=16 heads x 64, hidden 1024) on
8 Trainium2 NeuronCores.

Sharding: tensor-parallel over heads, 2 heads per core. Each core receives the
full activation (pre-transposed to [hidden, S] layout), its 128-row slice of
wq/wk/wv (transposed) and 128-column slice of wo (transposed), computes
q/k/v projections + flash-style causal attention for its 2 heads, applies its
slice of the output projection, and writes a full-shape partial output. The
host sums the 8 partials (the TP all-reduce) to produce the final output.

Kernel layout notes:
  - scores are computed TRANSPOSED: ST[sk, sq] = kT_tile^T @ qT_block, so the
    softmax numerator exp() runs PSUM->SBUF on the scalar engine with no
    transposes of the probability matrix anywhere.
  - the softmax denominator comes for free from the PV matmul by augmenting
    v with a ones column (stationary operand [v | 1], M=65): output row 64
    accumulates sum_k exp(s).
  - normalization happens after the out-projection commutes it out:
    att = OT/l per head before mixing heads, done on PSUM eviction.
  - all matmuls run in float32r (TF32-like, 1 cycle/row at N>=256).
"""
import sys
sys.path.insert(0, "/opt/trn_rl_repo")

import numpy as np

import concourse.bass as bass
import concourse.mybir as mybir
import concourse.tile as tile
from concourse.bass_utils import run_bass_kernel_spmd

# ---------------------------------------------------------------- constants
S = 4096          # sequence length
HID = 1024        # hidden dim
NCORES = 8
HPC = 2           # heads per core
HD = 64           # head dim
EPC = HPC * HD    # 128 e-dims (head-concat) per core
SB = 512          # q-block width
NB = S // SB      # 8 q-blocks
NT = S // 128     # 32 k-tiles
GROUP = 3         # k-tiles per exp batch (3 psum banks)

F32 = mybir.dt.float32
F32R = mybir.dt.float32r

_MAX_WAITS = 1    # this walrus build allows a single sync-wait per instruction


def _split_waits(nc):
    """Hoist extra sync-waits onto inserted same-engine drain carriers."""
    n = 0
    for fn in nc.m.functions:
        for bb in fn.blocks:
            insts = bb.instructions
            i = 0
            while i < len(insts):
                inst = insts[i]
                si = inst.sync_info
                w = list(si.on_wait) if si is not None and si.on_wait else []
                if len(w) > _MAX_WAITS:
                    chunks = [w[j:j + _MAX_WAITS] for j in range(0, len(w), _MAX_WAITS)]
                    si.on_wait = chunks[-1]
                    for ch in chunks[:-1]:
                        d = mybir.InstDrain(name=f"{inst.name}_ws{n}", ins=[], outs=[])
                        d.engine = inst.engine
                        d.sync_info = mybir.SyncInfo(on_wait=ch, on_update=[])
                        insts.insert(i, d)
                        i += 1
                        n += 1
                i += 1
    return n


def _build_nc():
    nc = bass.Bass(target_bir_lowering=False)

    xT = nc.declare_dram_parameter("xT", [HID, S], F32R, isOutput=False)
    wqT = nc.declare_dram_parameter("wqT", [HID, EPC], F32R, isOutput=False)
    wkT = nc.declare_dram_parameter("wkT", [HID, EPC], F32R, isOutput=False)
    wvT = nc.declare_dram_parameter("wvT", [HID, EPC], F32R, isOutput=False)
    woT = nc.declare_dram_parameter("woT", [EPC, HID], F32R, isOutput=False)
    cmask = nc.declare_dram_parameter("cmask", [128, 4 * SB], F32R, isOutput=False)
    ones = nc.declare_dram_parameter("ones", [1, 128], F32R, isOutput=False)
    ident = nc.declare_dram_parameter("ident", [128, 128], F32R, isOutput=False)
    out = nc.declare_dram_parameter("out", [S, HID], F32, isOutput=True)

    KH = HID // 128  # 8 contraction chunks for projections

    with tile.TileContext(nc) as tc:
        with tc.tile_pool(name="const", bufs=1) as const, \
             tc.tile_pool(name="qk", bufs=1) as qk, \
             tc.tile_pool(name="xt", bufs=2) as xtp, \
             tc.tile_pool(name="vt", bufs=2) as vtp, \
             tc.tile_pool(name="pt", bufs=3) as ptp, \
             tc.tile_pool(name="att", bufs=2) as attp, \
             tc.tile_pool(name="osb", bufs=3) as osbp, \
             tc.tile_pool(name="rl", bufs=4) as rlp, \
             tc.tile_pool(name="bc", bufs=2) as bcp, \
             tc.tile_pool(name="ps", bufs=2, space="PSUM") as psp, \
             tc.tile_pool(name="ot", bufs=2, space="PSUM") as otp:

            # ---- constants / weights
            wq_sb = const.tile([128, KH, EPC], F32R, tag="wq")
            wk_sb = const.tile([128, KH, EPC], F32R, tag="wk")
            wv_sb = const.tile([128, KH, EPC], F32R, tag="wv")
            for w_d, w_s in ((wqT, wq_sb), (wkT, wk_sb), (wvT, wv_sb)):
                nc.sync.dma_start(out=w_s, in_=w_d.rearrange("(k p) m -> p k m", p=128))
            wo_sb = const.tile([EPC, HID], F32R, tag="wo")
            nc.sync.dma_start(out=wo_sb, in_=woT[:, :])
            cm_sb = const.tile([128, 4 * SB], F32R, tag="cm")
            nc.sync.dma_start(out=cm_sb, in_=cmask[:, :])
            ones_sb = const.tile([1, 128], F32R, tag="ones")
            nc.sync.dma_start(out=ones_sb, in_=ones[:, :])
            id_sb = const.tile([128, 128], F32R, tag="id")
            nc.sync.dma_start(out=id_sb, in_=ident[:, :])

            qT = qk.tile([128, S], F32R, tag="qT")   # [e(2 heads), s]
            kT = qk.tile([128, S], F32R, tag="kT")
            vbuf = qk.tile([128, HPC, NT, 65], F32R, tag="v")  # [sk, h, t, v|1]
            nc.vector.memset(
                vbuf.rearrange("p a b c -> p (a b c)").bitcast(F32), 1.0)

            for b in range(NB):
                sl = slice(b * SB, (b + 1) * SB)
                # ---------- projections for s-block b
                xt = xtp.tile([128, KH, SB], F32R, tag="xt")
                nc.sync.dma_start(
                    out=xt, in_=xT.rearrange("(k p) s -> p k s", p=128)[:, :, sl])
                for w_s, dst in ((wq_sb, qT), (wk_sb, kT)):
                    ps = psp.tile([128, SB], F32, tag="st")
                    for k in range(KH):
                        nc.tensor.matmul(ps, w_s[:, k, :], xt[:, k, :],
                                         start=(k == 0), stop=(k == KH - 1))
                    nc.vector.tensor_copy(out=dst[:, sl], in_=ps)
                ps = psp.tile([128, SB], F32, tag="st")
                for k in range(KH):
                    nc.tensor.matmul(ps, wv_sb[:, k, :], xt[:, k, :],
                                     start=(k == 0), stop=(k == KH - 1))
                vt = vtp.tile([128, SB], F32R, tag="vt")
                nc.vector.tensor_copy(out=vt, in_=ps)
                for j in range(4):  # flip vT -> v (natural layout), per 128-tile
                    t = 4 * b + j
                    fp = psp.tile([128, 128], F32, tag="st")
                    nc.tensor.matmul(fp, vt[:, j * 128:(j + 1) * 128], id_sb,
                                     start=True, stop=True)
                    nc.vector.tensor_copy(out=vbuf[:, 0, t, 0:64], in_=fp[:, 0:64])
                    nc.vector.tensor_copy(out=vbuf[:, 1, t, 0:64], in_=fp[:, 64:128])

                # ---------- attention for q-block b (both heads)
                ntl = 4 * (b + 1)  # causal k-tiles
                ots = [otp.tile([65, SB], F32, tag="ot", name=f"ot{b}_{h}")
                       for h in range(HPC)]
                groups = [list(range(g, min(g + GROUP, ntl)))
                          for g in range(0, ntl, GROUP)]
                for grp in groups:
                    for h in range(HPC):
                        hsl = slice(64 * h, 64 * (h + 1))
                        st = psp.tile([128, GROUP * SB], F32, tag="st")
                        for i, t in enumerate(grp):
                            nc.tensor.matmul(
                                st[:, i * SB:(i + 1) * SB],
                                kT[hsl, t * 128:(t + 1) * 128],
                                qT[hsl, sl], start=True, stop=True)
                        L = len(grp) * SB
                        pt = ptp.tile([128, GROUP * SB], F32R, tag="pt")
                        nc.scalar.activation(out=pt[:, :L], in_=st[:, :L],
                                             func=mybir.ActivationFunctionType.Exp,
                                             scale=float(HD) ** -0.5)
                        for i, t in enumerate(grp):
                            j = t - 4 * b
                            if j >= 0:  # diagonal tile -> causal mask
                                psl = slice(i * SB, (i + 1) * SB)
                                nc.vector.tensor_mul(pt[:, psl], pt[:, psl],
                                                     cm_sb[:, j * SB:(j + 1) * SB])
                        for i, t in enumerate(grp):
                            nc.tensor.matmul(
                                ots[h], vbuf[:, h, t, :],
                                pt[:, i * SB:(i + 1) * SB],
                                start=(t == 0), stop=(t == ntl - 1))

                # ---------- normalize + merge heads: att[e, sq] = OT/l
                att = attp.tile([128, SB], F32R, tag="att")
                for h in range(HPC):
                    rl = rlp.tile([1, SB], F32R, tag="rl")
                    with nc.allow_low_precision(reason="f32r is bit-identical to f32 here"):
                        nc.vector.reciprocal(rl, ots[h][64:65, :])
                    bcps = psp.tile([64, SB], F32, tag="st")
                    nc.tensor.matmul(bcps, ones_sb[:, 0:64], rl,
                                     start=True, stop=True)
                    bc = bcp.tile([64, SB], F32, tag="bc")
                    nc.vector.tensor_copy(out=bc, in_=bcps)
                    nc.vector.tensor_mul(att[64 * h:64 * (h + 1), :],
                                         ots[h][0:64, :], bc)

                # ---------- output projection (partial over this core's heads)
                for m in range(4):
                    osb = osbp.tile([128, HID], F32, tag="osb")
                    for n2 in range(2):
                        op = psp.tile([128, 512], F32, tag="st")
                        nc.tensor.matmul(op, att[:, m * 128:(m + 1) * 128],
                                         wo_sb[:, n2 * 512:(n2 + 1) * 512],
                                         start=True, stop=True)
                        nc.vector.tensor_copy(out=osb[:, n2 * 512:(n2 + 1) * 512],
                                              in_=op)
                    r0 = (4 * b + m) * 128
                    nc.sync.dma_start(out=out[r0:r0 + 128, :], in_=osb)

    _split_waits(nc)
    return nc


_cached = {}


def _get_nc():
    if "nc" not in _cached:
        _cached["nc"] = _build_nc()
    return _cached["nc"]


def kernel(x, wq, wk, wv, wo):
    x = np.asarray(x, dtype=np.float32)
    wq, wk, wv, wo = (np.asarray(a, dtype=np.float32) for a in (wq, wk, wv, wo))
    B = x.shape[0]
    assert x.shape == (B, S, HID)

    xT = np.ascontiguousarray(x[0].T)                      # [HID, S]
    # static causal masks for the 4 diagonal tile offsets
    p = np.arange(128)[:, None]
    i = np.arange(SB)[None, :]
    cm = np.concatenate([(p + 128 * j <= i) for j in range(4)],
                        axis=1).astype(np.float32)          # [128, 4*SB]
    ones = np.ones((1, 128), dtype=np.float32)
    ident = np.eye(128, dtype=np.float32)

    in_maps = []
    for c in range(NCORES):
        esl = slice(c * EPC, (c + 1) * EPC)
        in_maps.append({
            "xT": xT,
            "wqT": np.ascontiguousarray(wq[esl, :].T),      # [HID, EPC]
            "wkT": np.ascontiguousarray(wk[esl, :].T),
            "wvT": np.ascontiguousarray(wv[esl, :].T),
            "woT": np.ascontiguousarray(wo[:, esl].T),      # [EPC, HID]
            "cmask": cm,
            "ones": ones,
            "ident": ident,
        })

    nc = _get_nc()
    res = run_bass_kernel_spmd(nc, in_maps, core_ids=list(range(NCORES)))
    acc = res.results[0]["out"].astype(np.float32)
    for c in range(1, NCORES):
        acc = acc + res.results[c]["out"]
    return acc.reshape(B, S, HID)


if __name__ == "__main__":
    # smoke test against numpy reference
    rng = np.random.default_rng(0)
    x = rng.standard_normal((1, S, HID), dtype=np.float32)
    lim = float(np.sqrt(6.0 / (HID + 16 * HD)))
    wq, wk, wv, wo = (rng.uniform(-lim, lim, (1024, 1024)).astype(np.float32)
                      for _ in range(4))
    got = kernel(x=x, wq=wq, wk=wk, wv=wv, wo=wo)
    print("kernel output", got.shape, got.dtype, got.flat[:4])


# revision 7
# speedup vs baseline: 1.0944x; 1.0944x over previous
"""Causal multi-head attention (B=1, S=4096, H=16 heads x 64, hidden 1024) on
8 Trainium2 NeuronCores.

Sharding: tensor-parallel over heads, 2 heads per core. Each core receives the
full activation (pre-transposed to [hidden, S] layout), its 128-row slice of
wq/wk/wv (transposed) and 128-column slice of wo (transposed), computes
q/k/v projections + flash-style causal attention for its 2 heads, applies its
slice of the output projection, and writes a full-shape partial output. The
host sums the 8 partials (the TP all-reduce) to produce the final output.

Kernel layout notes:
  - scores are computed TRANSPOSED: ST[sk, sq] = kT_tile^T @ qT_block, so the
    softmax numerator exp() runs PSUM->SBUF on the scalar engine with no
    transposes of the probability matrix anywhere.
  - the softmax denominator comes for free from the PV matmul by augmenting
    v with a ones column (stationary operand [v | 1], M=65): output row 64
    accumulates sum_k exp(s).
  - normalization happens after the out-projection commutes it out:
    att = OT/l per head before mixing heads, done on PSUM eviction.
  - all matmuls run in float32r (TF32-like, 1 cycle/row at N>=256).
"""
import sys
sys.path.insert(0, "/opt/trn_rl_repo")

import numpy as np

import concourse.bass as bass
import concourse.mybir as mybir
import concourse.tile as tile
from concourse.bass_utils import run_bass_kernel_spmd

# ---------------------------------------------------------------- constants
S = 4096          # sequence length
HID = 1024        # hidden dim
NCORES = 8
HPC = 2           # heads per core
HD = 64           # head dim
EPC = HPC * HD    # 128 e-dims (head-concat) per core
SB = 512          # q-block width
NB = S // SB      # 8 q-blocks
NT = S // 128     # 32 k-tiles
GROUP = 3         # k-tiles per exp batch (3 psum banks)

F32 = mybir.dt.float32
F32R = mybir.dt.float32r
F16 = mybir.dt.float16
DT = F16  # matmul operand dtype

_MAX_WAITS = 1    # this walrus build allows a single sync-wait per instruction


def _split_waits(nc):
    """Hoist extra sync-waits onto inserted same-engine drain carriers."""
    n = 0
    for fn in nc.m.functions:
        for bb in fn.blocks:
            insts = bb.instructions
            i = 0
            while i < len(insts):
                inst = insts[i]
                si = inst.sync_info
                w = list(si.on_wait) if si is not None and si.on_wait else []
                if len(w) > _MAX_WAITS:
                    chunks = [w[j:j + _MAX_WAITS] for j in range(0, len(w), _MAX_WAITS)]
                    si.on_wait = chunks[-1]
                    for ch in chunks[:-1]:
                        d = mybir.InstDrain(name=f"{inst.name}_ws{n}", ins=[], outs=[])
                        d.engine = inst.engine
                        d.sync_info = mybir.SyncInfo(on_wait=ch, on_update=[])
                        insts.insert(i, d)
                        i += 1
                        n += 1
                i += 1
    return n


def _build_nc():
    nc = bass.Bass(target_bir_lowering=False)

    xT = nc.declare_dram_parameter("xT", [HID, S], DT, isOutput=False)
    wqT = nc.declare_dram_parameter("wqT", [HID, EPC], DT, isOutput=False)
    wkT = nc.declare_dram_parameter("wkT", [HID, EPC], DT, isOutput=False)
    wvT = nc.declare_dram_parameter("wvT", [HID, EPC], DT, isOutput=False)
    woT = nc.declare_dram_parameter("woT", [EPC, HID], DT, isOutput=False)
    cmask = nc.declare_dram_parameter("cmask", [128, 4 * SB], DT, isOutput=False)
    ones = nc.declare_dram_parameter("ones", [1, 128], DT, isOutput=False)
    ident = nc.declare_dram_parameter("ident", [128, 128], DT, isOutput=False)
    out = nc.declare_dram_parameter("out", [S, HID], F32, isOutput=True)

    KH = HID // 128  # 8 contraction chunks for projections

    with tile.TileContext(nc) as tc:
        with tc.tile_pool(name="const", bufs=1) as const, \
             tc.tile_pool(name="qk", bufs=1) as qk, \
             tc.tile_pool(name="xt", bufs=2) as xtp, \
             tc.tile_pool(name="vt", bufs=2) as vtp, \
             tc.tile_pool(name="pt", bufs=3) as ptp, \
             tc.tile_pool(name="att", bufs=2) as attp, \
             tc.tile_pool(name="osb", bufs=3) as osbp, \
             tc.tile_pool(name="rl", bufs=4) as rlp, \
             tc.tile_pool(name="bc", bufs=2) as bcp, \
             tc.tile_pool(name="ps", bufs=2, space="PSUM") as psp, \
             tc.tile_pool(name="ot", bufs=2, space="PSUM") as otp:

            # ---- constants / weights
            wq_sb = const.tile([128, KH, EPC], DT, tag="wq")
            wk_sb = const.tile([128, KH, EPC], DT, tag="wk")
            wv_sb = const.tile([128, KH, EPC], DT, tag="wv")
            for w_d, w_s in ((wqT, wq_sb), (wkT, wk_sb), (wvT, wv_sb)):
                nc.sync.dma_start(out=w_s, in_=w_d.rearrange("(k p) m -> p k m", p=128))
            wo_sb = const.tile([EPC, HID], DT, tag="wo")
            nc.sync.dma_start(out=wo_sb, in_=woT[:, :])
            cm_sb = const.tile([128, 4 * SB], DT, tag="cm")
            nc.sync.dma_start(out=cm_sb, in_=cmask[:, :])
            ones_sb = const.tile([1, 128], DT, tag="ones")
            nc.sync.dma_start(out=ones_sb, in_=ones[:, :])
            id_sb = const.tile([128, 128], DT, tag="id")
            nc.sync.dma_start(out=id_sb, in_=ident[:, :])

            qT = qk.tile([128, S], DT, tag="qT")   # [e(2 heads), s]
            kT = qk.tile([128, S], DT, tag="kT")
            vbuf = qk.tile([128, HPC, NT, 65], DT, tag="v")  # [sk, h, t, v|1]
            nc.vector.memset(
                vbuf.rearrange("p a b c -> p (a b c)"), 1.0)

            for b in range(NB):
                sl = slice(b * SB, (b + 1) * SB)
                # ---------- projections for s-block b
                xt = xtp.tile([128, KH, SB], DT, tag="xt")
                nc.sync.dma_start(
                    out=xt, in_=xT.rearrange("(k p) s -> p k s", p=128)[:, :, sl])
                for w_s, dst in ((wq_sb, qT), (wk_sb, kT)):
                    ps = psp.tile([128, SB], F32, tag="st")
                    for k in range(KH):
                        nc.tensor.matmul(ps, w_s[:, k, :], xt[:, k, :],
                                         start=(k == 0), stop=(k == KH - 1))
                    nc.vector.tensor_copy(out=dst[:, sl], in_=ps)
                ps = psp.tile([128, SB], F32, tag="st")
                for k in range(KH):
                    nc.tensor.matmul(ps, wv_sb[:, k, :], xt[:, k, :],
                                     start=(k == 0), stop=(k == KH - 1))
                vt = vtp.tile([128, SB], DT, tag="vt")
                nc.vector.tensor_copy(out=vt, in_=ps)
                for j in range(4):  # flip vT -> v (natural layout), per 128-tile
                    t = 4 * b + j
                    fp = psp.tile([128, 128], F32, tag="st")
                    nc.tensor.matmul(fp, vt[:, j * 128:(j + 1) * 128], id_sb,
                                     start=True, stop=True)
                    nc.vector.tensor_copy(out=vbuf[:, 0, t, 0:64], in_=fp[:, 0:64])
                    nc.vector.tensor_copy(out=vbuf[:, 1, t, 0:64], in_=fp[:, 64:128])

                # ---------- attention for q-block b (both heads)
                ntl = 4 * (b + 1)  # causal k-tiles
                ots = [otp.tile([65, SB], F32, tag="ot", name=f"ot{b}_{h}")
                       for h in range(HPC)]
                groups = [list(range(g, min(g + GROUP, ntl)))
                          for g in range(0, ntl, GROUP)]
                for grp in groups:
                    for h in range(HPC):
                        hsl = slice(64 * h, 64 * (h + 1))
                        st = psp.tile([128, GROUP * SB], F32, tag="st")
                        for i, t in enumerate(grp):
                            nc.tensor.matmul(
                                st[:, i * SB:(i + 1) * SB],
                                kT[hsl, t * 128:(t + 1) * 128],
                                qT[hsl, sl], start=True, stop=True)
                        L = len(grp) * SB
                        pt = ptp.tile([128, GROUP * SB], DT, tag="pt")
                        nc.scalar.activation(out=pt[:, :L], in_=st[:, :L],
                                             func=mybir.ActivationFunctionType.Exp,
                                             scale=float(HD) ** -0.5)
                        for i, t in enumerate(grp):
                            j = t - 4 * b
                            if j >= 0:  # diagonal tile -> causal mask
                                psl = slice(i * SB, (i + 1) * SB)
                                nc.vector.tensor_mul(pt[:, psl], pt[:, psl],
                                                     cm_sb[:, j * SB:(j + 1) * SB])
                        for i, t in enumerate(grp):
                            nc.tensor.matmul(
                                ots[h], vbuf[:, h, t, :],
                                pt[:, i * SB:(i + 1) * SB],
                                start=(t == 0), stop=(t == ntl - 1))

                # ---------- normalize + merge heads: att[e, sq] = OT/l
                att = attp.tile([128, SB], DT, tag="att")
                for h in range(HPC):
                    rl = rlp.tile([1, SB], DT, tag="rl")
                    with nc.allow_low_precision(reason="f32r is bit-identical to f32 here"):
                        nc.vector.reciprocal(rl, ots[h][64:65, :])
                    bcps = psp.tile([64, SB], F32, tag="st")
                    nc.tensor.matmul(bcps, ones_sb[:, 0:64], rl,
                                     start=True, stop=True)
                    bc = bcp.tile([64, SB], F32, tag="bc")
                    nc.vector.tensor_copy(out=bc, in_=bcps)
                    nc.vector.tensor_mul(att[64 * h:64 * (h + 1), :],
                                         ots[h][0:64, :], bc)

                # ---------- output projection (partial over this core's heads)
                for m in range(4):
                    osb = osbp.tile([128, HID], F32, tag="osb")
                    for n2 in range(2):
                        op = psp.tile([128, 512], F32, tag="st")
                        nc.tensor.matmul(op, att[:, m * 128:(m + 1) * 128],
                                         wo_sb[:, n2 * 512:(n2 + 1) * 512],
                                         start=True, stop=True)
                        nc.vector.tensor_copy(out=osb[:, n2 * 512:(n2 + 1) * 512],
                                              in_=op)
                    r0 = (4 * b + m) * 128
                    nc.sync.dma_start(out=out[r0:r0 + 128, :], in_=osb)

    _split_waits(nc)
    return nc


_cached = {}


def _get_nc():
    if "nc" not in _cached:
        _cached["nc"] = _build_nc()
    return _cached["nc"]


def make_in_maps(x, wq, wk, wv, wo):
    x = np.asarray(x, dtype=np.float32)
    wq, wk, wv, wo = (np.asarray(a, dtype=np.float32) for a in (wq, wk, wv, wo))
    B = x.shape[0]
    assert x.shape == (B, S, HID)

    dt = np.float16
    xT = np.ascontiguousarray(x[0].T.astype(dt))            # [HID, S]
    # static causal masks for the 4 diagonal tile offsets
    p = np.arange(128)[:, None]
    i = np.arange(SB)[None, :]
    cm = np.concatenate([(p + 128 * j <= i) for j in range(4)],
                        axis=1).astype(dt)                  # [128, 4*SB]
    ones = np.ones((1, 128), dtype=dt)
    ident = np.eye(128, dtype=dt)

    in_maps = []
    for c in range(NCORES):
        esl = slice(c * EPC, (c + 1) * EPC)
        in_maps.append({
            "xT": xT,
            "wqT": np.ascontiguousarray(wq[esl, :].T.astype(dt)),
            "wkT": np.ascontiguousarray(wk[esl, :].T.astype(dt)),
            "wvT": np.ascontiguousarray(wv[esl, :].T.astype(dt)),
            "woT": np.ascontiguousarray(wo[:, esl].T.astype(dt)),
            "cmask": cm,
            "ones": ones,
            "ident": ident,
        })
    return in_maps


def kernel(x, wq, wk, wv, wo):
    B = np.asarray(x).shape[0]
    in_maps = make_in_maps(x, wq, wk, wv, wo)
    nc = _get_nc()
    res = run_bass_kernel_spmd(nc, in_maps, core_ids=list(range(NCORES)))
    acc = res.results[0]["out"].astype(np.float32)
    for c in range(1, NCORES):
        acc = acc + res.results[c]["out"]
    return acc.reshape(B, S, HID)


if __name__ == "__main__":
    # smoke test against numpy reference
    rng = np.random.default_rng(0)
    x = rng.standard_normal((1, S, HID), dtype=np.float32)
    lim = float(np.sqrt(6.0 / (HID + 16 * HD)))
    wq, wk, wv, wo = (rng.uniform(-lim, lim, (1024, 1024)).astype(np.float32)
                      for _ in range(4))
    got = kernel(x=x, wq=wq, wk=wk, wv=wv, wo=wo)
    print("kernel output", got.shape, got.dtype, got.flat[:4])


# revision 8
# speedup vs baseline: 1.1012x; 1.0062x over previous
"""Causal multi-head attention (B=1, S=4096, H=16 heads x 64, hidden 1024) on
8 Trainium2 NeuronCores.

Sharding: tensor-parallel over heads, 2 heads per core. Each core receives the
full activation (pre-transposed to [hidden, S] layout), its 128-row slice of
wq/wk/wv (transposed) and 128-column slice of wo (transposed), computes
q/k/v projections + flash-style causal attention for its 2 heads, applies its
slice of the output projection, and writes a full-shape partial output. The
host sums the 8 partials (the TP all-reduce) to produce the final output.

Kernel layout notes:
  - scores are computed TRANSPOSED: ST[sk, sq] = kT_tile^T @ qT_block, so the
    softmax numerator exp() runs PSUM->SBUF on the scalar engine with no
    transposes of the probability matrix anywhere.
  - the softmax denominator comes for free from the PV matmul by augmenting
    v with a ones column (stationary operand [v | 1], M=65): output row 64
    accumulates sum_k exp(s).
  - normalization happens after the out-projection commutes it out:
    att = OT/l per head before mixing heads, done on PSUM eviction.
  - all matmuls run in float32r (TF32-like, 1 cycle/row at N>=256).
"""
import sys
sys.path.insert(0, "/opt/trn_rl_repo")

import numpy as np

import concourse.bass as bass
import concourse.mybir as mybir
import concourse.tile as tile
from concourse.bass_utils import run_bass_kernel_spmd

# ---------------------------------------------------------------- constants
S = 4096          # sequence length
HID = 1024        # hidden dim
NCORES = 8
HPC = 2           # heads per core
HD = 64           # head dim
EPC = HPC * HD    # 128 e-dims (head-concat) per core
SB = 512          # q-block width
NB = S // SB      # 8 q-blocks
NT = S // 128     # 32 k-tiles
GROUP = 3         # k-tiles per exp batch (3 psum banks)

F32 = mybir.dt.float32
F32R = mybir.dt.float32r
F16 = mybir.dt.float16
DT = F16  # matmul operand dtype

_MAX_WAITS = 1    # this walrus build allows a single sync-wait per instruction


def _split_waits(nc):
    """Hoist extra sync-waits onto inserted same-engine drain carriers."""
    n = 0
    for fn in nc.m.functions:
        for bb in fn.blocks:
            insts = bb.instructions
            i = 0
            while i < len(insts):
                inst = insts[i]
                si = inst.sync_info
                w = list(si.on_wait) if si is not None and si.on_wait else []
                if len(w) > _MAX_WAITS:
                    chunks = [w[j:j + _MAX_WAITS] for j in range(0, len(w), _MAX_WAITS)]
                    si.on_wait = chunks[-1]
                    for ch in chunks[:-1]:
                        # EventSemaphore carrier: pure wait, no pipeline flush
                        # (InstDrain stalls the engine pipe ~1.5us per use).
                        d = mybir.InstEventSemaphore(
                            name=f"{inst.name}_ws{n}", ins=[], outs=[])
                        d.engine = inst.engine
                        d.sync_info = mybir.SyncInfo(on_wait=ch, on_update=[])
                        insts.insert(i, d)
                        i += 1
                        n += 1
                i += 1
    return n


def _build_nc():
    nc = bass.Bass(target_bir_lowering=False)

    xT = nc.declare_dram_parameter("xT", [HID, S], DT, isOutput=False)
    wqT = nc.declare_dram_parameter("wqT", [HID, EPC], DT, isOutput=False)
    wkT = nc.declare_dram_parameter("wkT", [HID, EPC], DT, isOutput=False)
    wvT = nc.declare_dram_parameter("wvT", [HID, EPC], DT, isOutput=False)
    woT = nc.declare_dram_parameter("woT", [EPC, HID], DT, isOutput=False)
    cmask = nc.declare_dram_parameter("cmask", [128, 4 * SB], DT, isOutput=False)
    ones = nc.declare_dram_parameter("ones", [1, 128], DT, isOutput=False)
    ident = nc.declare_dram_parameter("ident", [128, 128], DT, isOutput=False)
    out = nc.declare_dram_parameter("out", [S, HID], F32, isOutput=True)

    KH = HID // 128  # 8 contraction chunks for projections

    with tile.TileContext(nc) as tc:
        with tc.tile_pool(name="const", bufs=1) as const, \
             tc.tile_pool(name="qk", bufs=1) as qk, \
             tc.tile_pool(name="xt", bufs=2) as xtp, \
             tc.tile_pool(name="vt", bufs=2) as vtp, \
             tc.tile_pool(name="pt", bufs=3) as ptp, \
             tc.tile_pool(name="att", bufs=2) as attp, \
             tc.tile_pool(name="osb", bufs=3) as osbp, \
             tc.tile_pool(name="rl", bufs=4) as rlp, \
             tc.tile_pool(name="bc", bufs=2) as bcp, \
             tc.tile_pool(name="ps", bufs=2, space="PSUM") as psp, \
             tc.tile_pool(name="ot", bufs=2, space="PSUM") as otp:

            # ---- constants / weights
            wq_sb = const.tile([128, KH, EPC], DT, tag="wq")
            wk_sb = const.tile([128, KH, EPC], DT, tag="wk")
            wv_sb = const.tile([128, KH, EPC], DT, tag="wv")
            for w_d, w_s in ((wqT, wq_sb), (wkT, wk_sb), (wvT, wv_sb)):
                nc.sync.dma_start(out=w_s, in_=w_d.rearrange("(k p) m -> p k m", p=128))
            wo_sb = const.tile([EPC, HID], DT, tag="wo")
            nc.sync.dma_start(out=wo_sb, in_=woT[:, :])
            cm_sb = const.tile([128, 4 * SB], DT, tag="cm")
            nc.sync.dma_start(out=cm_sb, in_=cmask[:, :])
            ones_sb = const.tile([1, 128], DT, tag="ones")
            nc.sync.dma_start(out=ones_sb, in_=ones[:, :])
            id_sb = const.tile([128, 128], DT, tag="id")
            nc.sync.dma_start(out=id_sb, in_=ident[:, :])

            qT = qk.tile([128, S], DT, tag="qT")   # [e(2 heads), s]
            kT = qk.tile([128, S], DT, tag="kT")
            vbuf = qk.tile([128, HPC, NT, 65], DT, tag="v")  # [sk, h, t, v|1]
            nc.vector.memset(
                vbuf.rearrange("p a b c -> p (a b c)"), 1.0)

            for b in range(NB):
                sl = slice(b * SB, (b + 1) * SB)
                # ---------- projections for s-block b
                xt = xtp.tile([128, KH, SB], DT, tag="xt")
                nc.sync.dma_start(
                    out=xt, in_=xT.rearrange("(k p) s -> p k s", p=128)[:, :, sl])
                for w_s, dst in ((wq_sb, qT), (wk_sb, kT)):
                    ps = psp.tile([128, SB], F32, tag="st")
                    for k in range(KH):
                        nc.tensor.matmul(ps, w_s[:, k, :], xt[:, k, :],
                                         start=(k == 0), stop=(k == KH - 1))
                    nc.vector.tensor_copy(out=dst[:, sl], in_=ps)
                ps = psp.tile([128, SB], F32, tag="st")
                for k in range(KH):
                    nc.tensor.matmul(ps, wv_sb[:, k, :], xt[:, k, :],
                                     start=(k == 0), stop=(k == KH - 1))
                vt = vtp.tile([128, SB], DT, tag="vt")
                nc.vector.tensor_copy(out=vt, in_=ps)
                for j in range(4):  # flip vT -> v (natural layout), per 128-tile
                    t = 4 * b + j
                    fp = psp.tile([128, 128], F32, tag="st")
                    nc.tensor.matmul(fp, vt[:, j * 128:(j + 1) * 128], id_sb,
                                     start=True, stop=True)
                    nc.vector.tensor_copy(out=vbuf[:, 0, t, 0:64], in_=fp[:, 0:64])
                    nc.vector.tensor_copy(out=vbuf[:, 1, t, 0:64], in_=fp[:, 64:128])

                # ---------- attention for q-block b (both heads)
                ntl = 4 * (b + 1)  # causal k-tiles
                ots = [otp.tile([65, SB], F32, tag="ot", name=f"ot{b}_{h}")
                       for h in range(HPC)]
                groups = [list(range(g, min(g + GROUP, ntl)))
                          for g in range(0, ntl, GROUP)]
                for grp in groups:
                    for h in range(HPC):
                        hsl = slice(64 * h, 64 * (h + 1))
                        st = psp.tile([128, GROUP * SB], F32, tag="st")
                        for i, t in enumerate(grp):
                            nc.tensor.matmul(
                                st[:, i * SB:(i + 1) * SB],
                                kT[hsl, t * 128:(t + 1) * 128],
                                qT[hsl, sl], start=True, stop=True)
                        L = len(grp) * SB
                        pt = ptp.tile([128, GROUP * SB], DT, tag="pt")
                        nc.scalar.activation(out=pt[:, :L], in_=st[:, :L],
                                             func=mybir.ActivationFunctionType.Exp,
                                             scale=float(HD) ** -0.5)
                        for i, t in enumerate(grp):
                            j = t - 4 * b
                            if j >= 0:  # diagonal tile -> causal mask
                                psl = slice(i * SB, (i + 1) * SB)
                                nc.vector.tensor_mul(pt[:, psl], pt[:, psl],
                                                     cm_sb[:, j * SB:(j + 1) * SB])
                        for i, t in enumerate(grp):
                            nc.tensor.matmul(
                                ots[h], vbuf[:, h, t, :],
                                pt[:, i * SB:(i + 1) * SB],
                                start=(t == 0), stop=(t == ntl - 1))

                # ---------- normalize + merge heads: att[e, sq] = OT/l
                att = attp.tile([128, SB], DT, tag="att")
                for h in range(HPC):
                    rl = rlp.tile([1, SB], DT, tag="rl")
                    with nc.allow_low_precision(reason="f32r is bit-identical to f32 here"):
                        nc.vector.reciprocal(rl, ots[h][64:65, :])
                    bcps = psp.tile([64, SB], F32, tag="st")
                    nc.tensor.matmul(bcps, ones_sb[:, 0:64], rl,
                                     start=True, stop=True)
                    bc = bcp.tile([64, SB], F32, tag="bc")
                    nc.vector.tensor_copy(out=bc, in_=bcps)
                    nc.vector.tensor_mul(att[64 * h:64 * (h + 1), :],
                                         ots[h][0:64, :], bc)

                # ---------- output projection (partial over this core's heads)
                for m in range(4):
                    osb = osbp.tile([128, HID], F32, tag="osb")
                    for n2 in range(2):
                        op = psp.tile([128, 512], F32, tag="st")
                        nc.tensor.matmul(op, att[:, m * 128:(m + 1) * 128],
                                         wo_sb[:, n2 * 512:(n2 + 1) * 512],
                                         start=True, stop=True)
                        nc.vector.tensor_copy(out=osb[:, n2 * 512:(n2 + 1) * 512],
                                              in_=op)
                    r0 = (4 * b + m) * 128
                    nc.sync.dma_start(out=out[r0:r0 + 128, :], in_=osb)

    _split_waits(nc)
    return nc


_cached = {}


def _get_nc():
    if "nc" not in _cached:
        _cached["nc"] = _build_nc()
    return _cached["nc"]


def make_in_maps(x, wq, wk, wv, wo):
    x = np.asarray(x, dtype=np.float32)
    wq, wk, wv, wo = (np.asarray(a, dtype=np.float32) for a in (wq, wk, wv, wo))
    B = x.shape[0]
    assert x.shape == (B, S, HID)

    dt = np.float16
    xT = np.ascontiguousarray(x[0].T.astype(dt))            # [HID, S]
    # static causal masks for the 4 diagonal tile offsets
    p = np.arange(128)[:, None]
    i = np.arange(SB)[None, :]
    cm = np.concatenate([(p + 128 * j <= i) for j in range(4)],
                        axis=1).astype(dt)                  # [128, 4*SB]
    ones = np.ones((1, 128), dtype=dt)
    ident = np.eye(128, dtype=dt)

    in_maps = []
    for c in range(NCORES):
        esl = slice(c * EPC, (c + 1) * EPC)
        in_maps.append({
            "xT": xT,
            "wqT": np.ascontiguousarray(wq[esl, :].T.astype(dt)),
            "wkT": np.ascontiguousarray(wk[esl, :].T.astype(dt)),
            "wvT": np.ascontiguousarray(wv[esl, :].T.astype(dt)),
            "woT": np.ascontiguousarray(wo[:, esl].T.astype(dt)),
            "cmask": cm,
            "ones": ones,
            "ident": ident,
        })
    return in_maps


def kernel(x, wq, wk, wv, wo):
    B = np.asarray(x).shape[0]
    in_maps = make_in_maps(x, wq, wk, wv, wo)
    nc = _get_nc()
    res = run_bass_kernel_spmd(nc, in_maps, core_ids=list(range(NCORES)))
    acc = res.results[0]["out"].astype(np.float32)
    for c in range(1, NCORES):
        acc = acc + res.results[c]["out"]
    return acc.reshape(B, S, HID)


if __name__ == "__main__":
    # smoke test against numpy reference
    rng = np.random.default_rng(0)
    x = rng.standard_normal((1, S, HID), dtype=np.float32)
    lim = float(np.sqrt(6.0 / (HID + 16 * HD)))
    wq, wk, wv, wo = (rng.uniform(-lim, lim, (1024, 1024)).astype(np.float32)
                      for _ in range(4))
    got = kernel(x=x, wq=wq, wk=wk, wv=wv, wo=wo)
    print("kernel output", got.shape, got.dtype, got.flat[:4])


# revision 12
# speedup vs baseline: 1.1368x; 1.0324x over previous
"""Causal multi-head attention (B=1, S=4096, H=16 heads x 64, hidden 1024) on
8 Trainium2 NeuronCores.

Sharding: tensor-parallel over heads, 2 heads per core. Each core receives the
full activation (pre-transposed to [hidden, S] layout), its 128-row slice of
wq/wk/wv (transposed) and 128-column slice of wo (transposed), computes
q/k/v projections + flash-style causal attention for its 2 heads, applies its
slice of the output projection, and writes a full-shape partial output. The
host sums the 8 partials (the TP all-reduce) to produce the final output.

Kernel layout notes:
  - scores are computed TRANSPOSED: ST[sk, sq] = kT_tile^T @ qT_block, so the
    softmax numerator exp() runs PSUM->SBUF on the scalar engine with no
    transposes of the probability matrix anywhere.
  - the softmax denominator comes for free from the PV matmul by augmenting
    v with a ones column (stationary operand [v | 1], M=65): output row 64
    accumulates sum_k exp(s).
  - normalization happens after the out-projection commutes it out:
    att = OT/l per head before mixing heads, done on PSUM eviction.
  - all matmuls run in float32r (TF32-like, 1 cycle/row at N>=256).
"""
import sys
sys.path.insert(0, "/opt/trn_rl_repo")

import numpy as np

import concourse.bass as bass
import concourse.mybir as mybir
import concourse.tile as tile
from concourse.bass_utils import run_bass_kernel_spmd

# ---------------------------------------------------------------- constants
S = 4096          # sequence length
HID = 1024        # hidden dim
NCORES = 8
HPC = 2           # heads per core
HD = 64           # head dim
EPC = HPC * HD    # 128 e-dims (head-concat) per core
SB = 512          # q-block width
NB = S // SB      # 8 q-blocks
NT = S // 128     # 32 k-tiles
GROUP = 3         # k-tiles per exp batch (3 psum banks)

F32 = mybir.dt.float32
F32R = mybir.dt.float32r
F16 = mybir.dt.float16
DT = F16  # matmul operand dtype

_MAX_WAITS = 1    # this walrus build allows a single sync-wait per instruction


def _split_waits(nc):
    """Hoist extra sync-waits onto inserted same-engine drain carriers."""
    n = 0
    for fn in nc.m.functions:
        for bb in fn.blocks:
            insts = bb.instructions
            i = 0
            while i < len(insts):
                inst = insts[i]
                si = inst.sync_info
                w = list(si.on_wait) if si is not None and si.on_wait else []
                if len(w) > _MAX_WAITS:
                    chunks = [w[j:j + _MAX_WAITS] for j in range(0, len(w), _MAX_WAITS)]
                    si.on_wait = chunks[-1]
                    for ch in chunks[:-1]:
                        # EventSemaphore carrier: pure wait, no pipeline flush
                        # (InstDrain stalls the engine pipe ~1.5us per use).
                        d = mybir.InstEventSemaphore(
                            name=f"{inst.name}_ws{n}", ins=[], outs=[])
                        d.engine = inst.engine
                        d.sync_info = mybir.SyncInfo(on_wait=ch, on_update=[])
                        insts.insert(i, d)
                        i += 1
                        n += 1
                i += 1
    return n


def _build_nc():
    nc = bass.Bass(target_bir_lowering=False)

    xT = nc.declare_dram_parameter("xT", [HID, S], DT, isOutput=False)
    wqT = nc.declare_dram_parameter("wqT", [HID, EPC], DT, isOutput=False)
    wkT = nc.declare_dram_parameter("wkT", [HID, EPC], DT, isOutput=False)
    wvT = nc.declare_dram_parameter("wvT", [HID, EPC], DT, isOutput=False)
    woT = nc.declare_dram_parameter("woT", [EPC, HID], DT, isOutput=False)
    cmask = nc.declare_dram_parameter("cmask", [128, 4 * SB], DT, isOutput=False)
    ones = nc.declare_dram_parameter("ones", [1, 128], DT, isOutput=False)
    ident = nc.declare_dram_parameter("ident", [128, 128], DT, isOutput=False)
    out = nc.declare_dram_parameter("out", [S, HID], F32, isOutput=True)

    KH = HID // 128  # 8 contraction chunks for projections

    with tile.TileContext(nc) as tc:
        with tc.tile_pool(name="const", bufs=1) as const, \
             tc.tile_pool(name="qk", bufs=1) as qk, \
             tc.tile_pool(name="xt", bufs=2) as xtp, \
             tc.tile_pool(name="vt", bufs=2) as vtp, \
             tc.tile_pool(name="pt", bufs=3) as ptp, \
             tc.tile_pool(name="att", bufs=2) as attp, \
             tc.tile_pool(name="osb", bufs=3) as osbp, \
             tc.tile_pool(name="rl", bufs=4) as rlp, \
             tc.tile_pool(name="bc", bufs=2) as bcp, \
             tc.tile_pool(name="ps", bufs=2, space="PSUM") as psp, \
             tc.tile_pool(name="ot", bufs=2, space="PSUM") as otp:

            # ---- constants / weights
            wq_sb = const.tile([128, KH, EPC], DT, tag="wq")
            wk_sb = const.tile([128, KH, EPC], DT, tag="wk")
            wv_sb = const.tile([128, KH, EPC], DT, tag="wv")
            for w_d, w_s in ((wqT, wq_sb), (wkT, wk_sb), (wvT, wv_sb)):
                nc.sync.dma_start(out=w_s, in_=w_d.rearrange("(k p) m -> p k m", p=128))
            wo_sb = const.tile([EPC, HID], DT, tag="wo")
            nc.sync.dma_start(out=wo_sb, in_=woT[:, :])
            cm_sb = const.tile([128, 4 * SB], DT, tag="cm")
            nc.sync.dma_start(out=cm_sb, in_=cmask[:, :])
            ones_sb = const.tile([1, 128], DT, tag="ones")
            nc.sync.dma_start(out=ones_sb, in_=ones[:, :])
            id_sb = const.tile([128, 128], DT, tag="id")
            nc.sync.dma_start(out=id_sb, in_=ident[:, :])

            qT = qk.tile([128, S], DT, tag="qT")   # [e(2 heads), s]
            kT = qk.tile([128, S], DT, tag="kT")
            vbuf = qk.tile([128, HPC, NT, 65], DT, tag="v")  # [sk, h, t, v|1]
            nc.vector.memset(
                vbuf.rearrange("p a b c -> p (a b c)"), 1.0)

            def normalize_outproj(b, ots):
                sl = slice(b * SB, (b + 1) * SB)
                # normalize + merge heads: att[e, sq] = OT/l
                att = attp.tile([128, SB], DT, tag="att", name=f"att{b}")
                for h in range(HPC):
                    # l row PSUM->SBUF via ACT (single-partition DVE is slow),
                    # broadcast on PE, then full-width reciprocal on DVE.
                    lrow = rlp.tile([1, SB], DT, tag="rl", name=f"rl{b}_{h}")
                    nc.scalar.copy(out=lrow, in_=ots[h][64:65, :])
                    bcps = psp.tile([64, SB], F32, tag="st", name=f"bc{b}_{h}")
                    nc.tensor.matmul(bcps, ones_sb[:, 0:64], lrow,
                                     start=True, stop=True)
                    bc = bcp.tile([64, SB], F32, tag="bc", name=f"bcs{b}_{h}")
                    with nc.allow_low_precision(reason="normalization factor"):
                        nc.vector.reciprocal(bc, bcps)
                    nc.vector.tensor_mul(att[64 * h:64 * (h + 1), :],
                                         ots[h][0:64, :], bc)
                # output projection (partial over this core's heads)
                for m in range(4):
                    osb = osbp.tile([128, HID], F32, tag="osb", name=f"osb{b}_{m}")
                    for n2 in range(2):
                        op = psp.tile([128, 512], F32, tag="st", name=f"op{b}_{m}_{n2}")
                        nc.tensor.matmul(op, att[:, m * 128:(m + 1) * 128],
                                         wo_sb[:, n2 * 512:(n2 + 1) * 512],
                                         start=True, stop=True)
                        nc.vector.tensor_copy(out=osb[:, n2 * 512:(n2 + 1) * 512],
                                              in_=op)
                    r0 = (4 * b + m) * 128
                    nc.sync.dma_start(out=out[r0:r0 + 128, :], in_=osb)

            pending_no = None   # (b, ots) awaiting normalize + out-projection
            for b in range(NB):
                sl = slice(b * SB, (b + 1) * SB)
                # ---------- projections for s-block b
                xt = xtp.tile([128, KH, SB], DT, tag="xt")
                nc.sync.dma_start(
                    out=xt, in_=xT.rearrange("(k p) s -> p k s", p=128)[:, :, sl])
                for w_s, dst in ((wq_sb, qT), (wk_sb, kT)):
                    ps = psp.tile([128, SB], F32, tag="st")
                    for k in range(KH):
                        nc.tensor.matmul(ps, w_s[:, k, :], xt[:, k, :],
                                         start=(k == 0), stop=(k == KH - 1))
                    nc.vector.tensor_copy(out=dst[:, sl], in_=ps)
                ps = psp.tile([128, SB], F32, tag="st")
                for k in range(KH):
                    nc.tensor.matmul(ps, wv_sb[:, k, :], xt[:, k, :],
                                     start=(k == 0), stop=(k == KH - 1))
                vt = vtp.tile([128, SB], DT, tag="vt")
                nc.vector.tensor_copy(out=vt, in_=ps)
                for j in range(4):  # flip vT -> v (natural layout), per 128-tile
                    t = 4 * b + j
                    fp = psp.tile([128, 128], F32, tag="st")
                    nc.tensor.matmul(fp, vt[:, j * 128:(j + 1) * 128], id_sb,
                                     start=True, stop=True)
                    nc.vector.tensor_copy(out=vbuf[:, 0, t, 0:64], in_=fp[:, 0:64])
                    nc.vector.tensor_copy(out=vbuf[:, 1, t, 0:64], in_=fp[:, 64:128])

                # ---------- deferred normalize/out-proj of the previous block
                # (its latency hides under this block's projection matmuls)
                if pending_no is not None:
                    normalize_outproj(*pending_no)
                    pending_no = None

                # ---------- attention for q-block b (both heads)
                # software pipeline over (head, group) slots: emit ST(slot)
                # then PV(slot-1), so PV never stalls the PE on the exp.
                ntl = 4 * (b + 1)  # causal k-tiles
                ots = [otp.tile([65, SB], F32, tag="ot", name=f"ot{b}_{h}")
                       for h in range(HPC)]
                groups = [list(range(g, min(g + GROUP, ntl)))
                          for g in range(0, ntl, GROUP)]
                slots = [(h, grp) for grp in groups for h in range(HPC)]
                pend = []   # pending (h, grp, pt) awaiting PV emission

                def emit_pv(h, grp, pt):
                    for i, t in enumerate(grp):
                        nc.tensor.matmul(
                            ots[h], vbuf[:, h, t, :],
                            pt[:, i * SB:(i + 1) * SB],
                            start=(t == 0), stop=(t == ntl - 1))

                for h, grp in slots:
                    hsl = slice(64 * h, 64 * (h + 1))
                    st = psp.tile([128, GROUP * SB], F32, tag="st",
                                  name=f"st{b}_{h}_{grp[0]}")
                    for i, t in enumerate(grp):
                        nc.tensor.matmul(
                            st[:, i * SB:(i + 1) * SB],
                            kT[hsl, t * 128:(t + 1) * 128],
                            qT[hsl, sl], start=True, stop=True)
                    L = len(grp) * SB
                    pt = ptp.tile([128, GROUP * SB], DT, tag="pt",
                                  name=f"pt{b}_{h}_{grp[0]}")
                    nc.scalar.activation(out=pt[:, :L], in_=st[:, :L],
                                         func=mybir.ActivationFunctionType.Exp,
                                         scale=float(HD) ** -0.5)
                    for i, t in enumerate(grp):
                        j = t - 4 * b
                        if j >= 0:  # diagonal tile -> causal mask
                            psl = slice(i * SB, (i + 1) * SB)
                            nc.vector.tensor_mul(pt[:, psl], pt[:, psl],
                                                 cm_sb[:, j * SB:(j + 1) * SB])
                    pend.append((h, grp, pt))
                    if len(pend) > 1:
                        emit_pv(*pend.pop(0))
                emit_pv(*pend.pop(0))
                pending_no = (b, ots)

            normalize_outproj(*pending_no)

    _split_waits(nc)
    return nc


_cached = {}


def _get_nc():
    if "nc" not in _cached:
        _cached["nc"] = _build_nc()
    return _cached["nc"]


def make_in_maps(x, wq, wk, wv, wo):
    x = np.asarray(x, dtype=np.float32)
    wq, wk, wv, wo = (np.asarray(a, dtype=np.float32) for a in (wq, wk, wv, wo))
    B = x.shape[0]
    assert x.shape == (B, S, HID)

    dt = np.float16
    xT = np.ascontiguousarray(x[0].T.astype(dt))            # [HID, S]
    # static causal masks for the 4 diagonal tile offsets
    p = np.arange(128)[:, None]
    i = np.arange(SB)[None, :]
    cm = np.concatenate([(p + 128 * j <= i) for j in range(4)],
                        axis=1).astype(dt)                  # [128, 4*SB]
    ones = np.ones((1, 128), dtype=dt)
    ident = np.eye(128, dtype=dt)

    in_maps = []
    for c in range(NCORES):
        esl = slice(c * EPC, (c + 1) * EPC)
        in_maps.append({
            "xT": xT,
            "wqT": np.ascontiguousarray(wq[esl, :].T.astype(dt)),
            "wkT": np.ascontiguousarray(wk[esl, :].T.astype(dt)),
            "wvT": np.ascontiguousarray(wv[esl, :].T.astype(dt)),
            "woT": np.ascontiguousarray(wo[:, esl].T.astype(dt)),
            "cmask": cm,
            "ones": ones,
            "ident": ident,
        })
    return in_maps


def kernel(x, wq, wk, wv, wo):
    B = np.asarray(x).shape[0]
    in_maps = make_in_maps(x, wq, wk, wv, wo)
    nc = _get_nc()
    res = run_bass_kernel_spmd(nc, in_maps, core_ids=list(range(NCORES)))
    acc = res.results[0]["out"].astype(np.float32)
    for c in range(1, NCORES):
        acc = acc + res.results[c]["out"]
    return acc.reshape(B, S, HID)


if __name__ == "__main__":
    # smoke test against numpy reference
    rng = np.random.default_rng(0)
    x = rng.standard_normal((1, S, HID), dtype=np.float32)
    lim = float(np.sqrt(6.0 / (HID + 16 * HD)))
    wq, wk, wv, wo = (rng.uniform(-lim, lim, (1024, 1024)).astype(np.float32)
                      for _ in range(4))
    got = kernel(x=x, wq=wq, wk=wk, wv=wv, wo=wo)
    print("kernel output", got.shape, got.dtype, got.flat[:4])


# revision 17
# speedup vs baseline: 1.4575x; 1.2821x over previous
"""Causal multi-head attention (B=1, S=4096, H=16 heads x 64, hidden 1024) on
8 Trainium2 NeuronCores.

Sharding: tensor-parallel over heads, 2 heads per core. Each core receives the
full activation (pre-transposed to [hidden, S] layout), its 128-row slice of
wq/wk/wv (transposed) and 128-column slice of wo (transposed), computes
q/k/v projections + flash-style causal attention for its 2 heads, applies its
slice of the output projection, and writes a full-shape partial output. The
host sums the 8 partials (the TP all-reduce) to produce the final output.

Kernel layout notes:
  - scores are computed TRANSPOSED: ST[sk, sq] = kT_tile^T @ qT_block, so the
    softmax numerator exp() runs PSUM->SBUF on the scalar engine with no
    transposes of the probability matrix anywhere.
  - the softmax denominator comes for free from the PV matmul by augmenting
    v with a ones column (stationary operand [v | 1], M=65): output row 64
    accumulates sum_k exp(s).
  - normalization happens after the out-projection commutes it out:
    att = OT/l per head before mixing heads, done on PSUM eviction.
  - all matmuls run in float32r (TF32-like, 1 cycle/row at N>=256).
"""
import sys
sys.path.insert(0, "/opt/trn_rl_repo")

import numpy as np

import concourse.bass as bass
import concourse.mybir as mybir
import concourse.tile as tile
from concourse.bass_utils import run_bass_kernel_spmd

# ---------------------------------------------------------------- constants
S = 4096          # sequence length
HID = 1024        # hidden dim
NCORES = 8
HPC = 2           # heads per core
HD = 64           # head dim
EPC = HPC * HD    # 128 e-dims (head-concat) per core
SB = 512          # q-block width
NB = S // SB      # 8 q-blocks
NT = S // 128     # 32 k-tiles
GROUP = 3         # k-tiles per exp batch (3 psum banks)

F32 = mybir.dt.float32
F32R = mybir.dt.float32r
F16 = mybir.dt.float16
DT = F16  # matmul operand dtype

_MAX_WAITS = 1    # this walrus build allows a single sync-wait per instruction


def _split_waits(nc):
    """Hoist extra sync-waits onto inserted same-engine drain carriers."""
    n = 0
    for fn in nc.m.functions:
        for bb in fn.blocks:
            insts = bb.instructions
            i = 0
            while i < len(insts):
                inst = insts[i]
                si = inst.sync_info
                w = list(si.on_wait) if si is not None and si.on_wait else []
                if len(w) > _MAX_WAITS:
                    chunks = [w[j:j + _MAX_WAITS] for j in range(0, len(w), _MAX_WAITS)]
                    si.on_wait = chunks[-1]
                    for ch in chunks[:-1]:
                        # EventSemaphore carrier: pure wait, no pipeline flush
                        # (InstDrain stalls the engine pipe ~1.5us per use).
                        d = mybir.InstEventSemaphore(
                            name=f"{inst.name}_ws{n}", ins=[], outs=[])
                        d.engine = inst.engine
                        d.sync_info = mybir.SyncInfo(on_wait=ch, on_update=[])
                        insts.insert(i, d)
                        i += 1
                        n += 1
                i += 1
    return n


def _build_nc():
    nc = bass.Bass(target_bir_lowering=False)

    xT = nc.declare_dram_parameter("xT", [HID, S], DT, isOutput=False)
    wqT = nc.declare_dram_parameter("wqT", [HID, EPC], DT, isOutput=False)
    wkT = nc.declare_dram_parameter("wkT", [HID, EPC], DT, isOutput=False)
    wvT = nc.declare_dram_parameter("wvT", [HID, EPC], DT, isOutput=False)
    woT = nc.declare_dram_parameter("woT", [EPC, HID], DT, isOutput=False)
    cmask = nc.declare_dram_parameter("cmask", [128, 4 * SB], DT, isOutput=False)
    ones = nc.declare_dram_parameter("ones", [1, 128], DT, isOutput=False)
    ident = nc.declare_dram_parameter("ident", [128, 128], DT, isOutput=False)
    out = nc.declare_dram_parameter("out", [S, HID], F32, isOutput=True)

    KH = HID // 128  # 8 contraction chunks for projections

    with tile.TileContext(nc) as tc:
        with tc.tile_pool(name="const", bufs=1) as const, \
             tc.tile_pool(name="qk", bufs=1) as qk, \
             tc.tile_pool(name="xt", bufs=2) as xtp, \
             tc.tile_pool(name="vt", bufs=2) as vtp, \
             tc.tile_pool(name="pt", bufs=3) as ptp, \
             tc.tile_pool(name="att", bufs=2) as attp, \
             tc.tile_pool(name="osb", bufs=3) as osbp, \
             tc.tile_pool(name="rl", bufs=4) as rlp, \
             tc.tile_pool(name="bc", bufs=2) as bcp, \
             tc.tile_pool(name="ps", bufs=2, space="PSUM") as psp, \
             tc.tile_pool(name="ot", bufs=2, space="PSUM") as otp:

            # ---- constants / weights
            wq_sb = const.tile([128, KH, EPC], DT, tag="wq")
            wk_sb = const.tile([128, KH, EPC], DT, tag="wk")
            wv_sb = const.tile([128, KH, EPC], DT, tag="wv")
            for w_d, w_s in ((wqT, wq_sb), (wkT, wk_sb), (wvT, wv_sb)):
                nc.sync.dma_start(out=w_s, in_=w_d.rearrange("(k p) m -> p k m", p=128))
            wo_sb = const.tile([EPC, HID], DT, tag="wo")
            nc.sync.dma_start(out=wo_sb, in_=woT[:, :])
            cm_sb = const.tile([128, 4 * SB], DT, tag="cm")
            nc.sync.dma_start(out=cm_sb, in_=cmask[:, :])
            ones_sb = const.tile([1, 128], DT, tag="ones")
            nc.sync.dma_start(out=ones_sb, in_=ones[:, :])
            id_sb = const.tile([128, 128], DT, tag="id")
            nc.sync.dma_start(out=id_sb, in_=ident[:, :])

            qT = qk.tile([128, S], DT, tag="qT")   # [e(2 heads), s]
            kT = qk.tile([128, S], DT, tag="kT")
            vbuf = qk.tile([128, HPC, NT, 65], DT, tag="v")  # [sk, h, t, v|1]
            nc.vector.memset(
                vbuf.rearrange("p a b c -> p (a b c)"), 1.0)

            def make_deferred(b, ots):
                """Closures for block b's normalize + out-projection, emitted
                later (interleaved into the next block's attention slots) so
                the reciprocal/eviction latency hides under PE work."""
                sl = slice(b * SB, (b + 1) * SB)
                att = attp.tile([128, SB], DT, tag="att", name=f"att{b}")

                def norm_head(h):
                    # l row PSUM->SBUF via ACT (single-partition DVE is slow),
                    # broadcast on PE, then fast approx reciprocal on DVE.
                    lrow = rlp.tile([1, SB], DT, tag="rl", name=f"rl{b}_{h}")
                    nc.scalar.copy(out=lrow, in_=ots[h][64:65, :])
                    bcps = psp.tile([64, SB], F32, tag="st", name=f"bc{b}_{h}")
                    nc.tensor.matmul(bcps, ones_sb[:, 0:64], lrow,
                                     start=True, stop=True)
                    bc = bcp.tile([64, SB], F32, tag="bc", name=f"bcs{b}_{h}")
                    with nc.allow_low_precision(reason="normalization factor"):
                        nc.vector.reciprocal(bc, bcps)
                    nc.vector.tensor_mul(att[64 * h:64 * (h + 1), :],
                                         ots[h][0:64, :], bc)

                def outproj(mlist):
                    for m in mlist:
                        osb = osbp.tile([128, HID], F32, tag="osb",
                                        name=f"osb{b}_{m}")
                        for n2 in range(2):
                            op = psp.tile([128, 512], F32, tag="st",
                                          name=f"op{b}_{m}_{n2}")
                            nc.tensor.matmul(op, att[:, m * 128:(m + 1) * 128],
                                             wo_sb[:, n2 * 512:(n2 + 1) * 512],
                                             start=True, stop=True)
                            nc.vector.tensor_copy(
                                out=osb[:, n2 * 512:(n2 + 1) * 512], in_=op)
                        r0 = (4 * b + m) * 128
                        nc.sync.dma_start(out=out[r0:r0 + 128, :], in_=osb)

                return [lambda: norm_head(0), lambda: norm_head(1),
                        lambda: outproj([0, 1]), lambda: outproj([2, 3])]

            deferred = []   # emission closures from the previous block
            for b in range(NB):
                sl = slice(b * SB, (b + 1) * SB)
                # ---------- projections for s-block b
                xt = xtp.tile([128, KH, SB], DT, tag="xt")
                nc.sync.dma_start(
                    out=xt, in_=xT.rearrange("(k p) s -> p k s", p=128)[:, :, sl])
                # emit all projection matmuls before their evictions so the
                # DVE eviction latency hides under the next accumulation
                psq = psp.tile([128, SB], F32, tag="st", name=f"psq{b}")
                psk = psp.tile([128, SB], F32, tag="st", name=f"psk{b}")
                for k in range(KH):
                    nc.tensor.matmul(psq, wq_sb[:, k, :], xt[:, k, :],
                                     start=(k == 0), stop=(k == KH - 1))
                for k in range(KH):
                    nc.tensor.matmul(psk, wk_sb[:, k, :], xt[:, k, :],
                                     start=(k == 0), stop=(k == KH - 1))
                nc.vector.tensor_copy(out=qT[:, sl], in_=psq)
                psv = psp.tile([128, SB], F32, tag="st", name=f"psv{b}")
                for k in range(KH):
                    nc.tensor.matmul(psv, wv_sb[:, k, :], xt[:, k, :],
                                     start=(k == 0), stop=(k == KH - 1))
                nc.vector.tensor_copy(out=kT[:, sl], in_=psk)
                vt = vtp.tile([128, SB], DT, tag="vt")
                nc.vector.tensor_copy(out=vt, in_=psv)
                # flip vT -> v (natural layout) per 128-tile; keep one flip
                # matmul in flight ahead of its eviction
                fps = []

                def evict_flip(j, fp):
                    t = 4 * b + j
                    nc.vector.tensor_copy(out=vbuf[:, 0, t, 0:64], in_=fp[:, 0:64])
                    nc.vector.tensor_copy(out=vbuf[:, 1, t, 0:64], in_=fp[:, 64:128])

                for j in range(4):
                    fp = psp.tile([128, 128], F32, tag="st", name=f"fp{b}_{j}")
                    nc.tensor.matmul(fp, vt[:, j * 128:(j + 1) * 128], id_sb,
                                     start=True, stop=True)
                    fps.append((j, fp))
                    if len(fps) > 1:
                        evict_flip(*fps.pop(0))
                evict_flip(*fps.pop(0))

                # ---------- attention for q-block b (both heads)
                # software pipeline over (head, group) slots: emit ST(slot)
                # then PV(slot-1), so PV never stalls the PE on the exp.
                # The previous block's deferred normalize/out-proj closures are
                # drip-fed between slots so their latency hides under PE work.
                ntl = 4 * (b + 1)  # causal k-tiles
                ots = [otp.tile([65, SB], F32, tag="ot", name=f"ot{b}_{h}")
                       for h in range(HPC)]
                groups = [list(range(g, min(g + GROUP, ntl)))
                          for g in range(0, ntl, GROUP)]
                slots = [(h, grp) for grp in groups for h in range(HPC)]
                pend = []   # pending (h, grp, pt) awaiting PV emission

                def emit_pv(h, grp, pt):
                    for i, t in enumerate(grp):
                        nc.tensor.matmul(
                            ots[h], vbuf[:, h, t, :],
                            pt[:, i * SB:(i + 1) * SB],
                            start=(t == 0), stop=(t == ntl - 1))

                for h, grp in slots:
                    hsl = slice(64 * h, 64 * (h + 1))
                    st = psp.tile([128, GROUP * SB], F32, tag="st",
                                  name=f"st{b}_{h}_{grp[0]}")
                    for i, t in enumerate(grp):
                        nc.tensor.matmul(
                            st[:, i * SB:(i + 1) * SB],
                            kT[hsl, t * 128:(t + 1) * 128],
                            qT[hsl, sl], start=True, stop=True)
                    L = len(grp) * SB
                    pt = ptp.tile([128, GROUP * SB], DT, tag="pt",
                                  name=f"pt{b}_{h}_{grp[0]}")
                    nc.scalar.activation(out=pt[:, :L], in_=st[:, :L],
                                         func=mybir.ActivationFunctionType.Exp,
                                         scale=float(HD) ** -0.5)
                    for i, t in enumerate(grp):
                        j = t - 4 * b
                        if j >= 0:  # diagonal tile -> causal mask
                            psl = slice(i * SB, (i + 1) * SB)
                            nc.vector.tensor_mul(pt[:, psl], pt[:, psl],
                                                 cm_sb[:, j * SB:(j + 1) * SB])
                    pend.append((h, grp, pt))
                    if len(pend) > 1:
                        emit_pv(*pend.pop(0))
                    if deferred:
                        deferred.pop(0)()
                emit_pv(*pend.pop(0))
                while deferred:
                    deferred.pop(0)()
                deferred = make_deferred(b, ots)

            while deferred:
                deferred.pop(0)()

    _split_waits(nc)
    return nc


_cached = {}


def _get_nc():
    if "nc" not in _cached:
        _cached["nc"] = _build_nc()
    return _cached["nc"]


def make_in_maps(x, wq, wk, wv, wo):
    x = np.asarray(x, dtype=np.float32)
    wq, wk, wv, wo = (np.asarray(a, dtype=np.float32) for a in (wq, wk, wv, wo))
    B = x.shape[0]
    assert x.shape == (B, S, HID)

    dt = np.float16
    xT = np.ascontiguousarray(x[0].T.astype(dt))            # [HID, S]
    # static causal masks for the 4 diagonal tile offsets
    p = np.arange(128)[:, None]
    i = np.arange(SB)[None, :]
    cm = np.concatenate([(p + 128 * j <= i) for j in range(4)],
                        axis=1).astype(dt)                  # [128, 4*SB]
    ones = np.ones((1, 128), dtype=dt)
    ident = np.eye(128, dtype=dt)

    in_maps = []
    for c in range(NCORES):
        esl = slice(c * EPC, (c + 1) * EPC)
        in_maps.append({
            "xT": xT,
            "wqT": np.ascontiguousarray(wq[esl, :].T.astype(dt)),
            "wkT": np.ascontiguousarray(wk[esl, :].T.astype(dt)),
            "wvT": np.ascontiguousarray(wv[esl, :].T.astype(dt)),
            "woT": np.ascontiguousarray(wo[:, esl].T.astype(dt)),
            "cmask": cm,
            "ones": ones,
            "ident": ident,
        })
    return in_maps


def kernel(x, wq, wk, wv, wo):
    B = np.asarray(x).shape[0]
    in_maps = make_in_maps(x, wq, wk, wv, wo)
    nc = _get_nc()
    res = run_bass_kernel_spmd(nc, in_maps, core_ids=list(range(NCORES)))
    acc = res.results[0]["out"].astype(np.float32)
    for c in range(1, NCORES):
        acc = acc + res.results[c]["out"]
    return acc.reshape(B, S, HID)


if __name__ == "__main__":
    # smoke test against numpy reference
    rng = np.random.default_rng(0)
    x = rng.standard_normal((1, S, HID), dtype=np.float32)
    lim = float(np.sqrt(6.0 / (HID + 16 * HD)))
    wq, wk, wv, wo = (rng.uniform(-lim, lim, (1024, 1024)).astype(np.float32)
                      for _ in range(4))
    got = kernel(x=x, wq=wq, wk=wk, wv=wv, wo=wo)
    print("kernel output", got.shape, got.dtype, got.flat[:4])
